# revision 27
# baseline (speedup 1.0000x reference)
"""LightGCN message-passing + BPR loss on 8 Trainium2 NeuronCores.

Dest-sharded SpMM with pipelined AllGathers. Nodes are permuted into 8 core
shards (dealt round-robin by degree), each shard split into TWO row regions
(72 + 75 dest-windows of 128 rows). The inter-layer exchange is TWO
AllGather collectives (one per region) so each layer's compute overlaps the
other region's collective: every layer runs as region-phases — phase p of
any dest-window only gathers source embeddings from source-region p, so
phase p of layer l+1 depends only on AG chunk p of layer l.

Per 128-token chunk a selection matrix S[token, dest_local] =
val(token) * (dlocal(token) == dest_local) is built with one DVE
tensor_scalar op and matmul-accumulated into the dest window's PSUM tile
(one PSUM accumulation group per phase; phases summed via an SBUF ysum
tile). The BPR head computes the L2-reg term fully locally (each sampled
row is owned by exactly one core) followed by a tiny scalar AllReduce that
runs under the layer compute; only the pooled-embedding compact+AllGather
remains on the critical tail.

Returns (loss1, reg_loss) like the reference.
"""
import sys

sys.path.insert(0, "/opt/trn_rl_repo")

import numpy as np
import ml_dtypes

import concourse.bass as bass
import concourse.bacc as bacc
import concourse.tile as tile
from concourse import mybir, library_config

# ---------------- problem constants (hardcoded per spec) ----------------
NUM_USERS = 100000
NUM_ITEMS = 50000
DIM = 64
BATCH = 8192
NCORES = 8

P = 128                      # partitions / rows per dest window
NDW = 147                    # dest windows per core
SHARD = NDW * P              # 18816 rows per core
REG_DWS = (72, 75)           # dest windows per region
DW0 = (0, 72)                # first dw of each region
REG_ROWS = (REG_DWS[0] * P, REG_DWS[1] * P)            # 9216, 9600
REG_BASE = (0, REG_ROWS[0])                            # local row base
REG_GROWS = (NCORES * REG_ROWS[0], NCORES * REG_ROWS[1])  # 73728, 76800
NTOT_G = REG_GROWS[0] + REG_GROWS[1]                   # 150528
# source windows: 3 per region, int16-addressable
WIN_SIZE = (24576, 24576, 24576, 25600, 25600, 25600)
WIN_REG = (0, 0, 0, 1, 1, 1)
WIN_LBASE = (0, 24576, 49152, 0, 25600, 51200)         # base within region
LOCAL_W = 6                  # pseudo-window: source row owned by this core
NSW = 7
# phases per dw-chunk: phase 0 = local sources (no AG dependency),
# phase 1 = remote region-0 windows, phase 2 = remote region-1 windows
PHASES = ((LOCAL_W,), (0, 1, 2), (3, 4, 5))
# block emission order: both local phases first (they read shard_bounce,
# which ph2 blocks overwrite), then dwc0 remote (-> AG chunk 0), then dwc1
BLK_ORDER = ((0, 0), (1, 0), (0, 1), (0, 2), (1, 1), (1, 2))
SG = 8                       # dest windows per supergroup (PSUM banks)
MAXCH = 16                   # chunks (128 tokens each) per dma_gather

SG_BLOCKS = (
    [list(range(s, s + SG)) for s in range(0, REG_DWS[0], SG)],
    [list(range(s, min(s + SG, NDW))) for s in range(DW0[1], NDW, SG)],
)


# ---------------- host-side graph preprocessing ----------------
def _preprocess(edge_row, edge_col, edge_vals):
    n_nodes = NUM_USERS + NUM_ITEMS
    deg = np.bincount(edge_row, minlength=n_nodes)

    # Deal degree-sorted nodes round-robin over the 1176 (core, dw) slots in
    # snake order; partition lane = deal round.
    order = np.argsort(-deg, kind="stable")
    nslots = NCORES * NDW
    idx = np.arange(n_nodes)
    rounds = idx // nslots
    within = idx % nslots
    snake = np.where(rounds % 2 == 0, within, nslots - 1 - within)
    slot_ids = np.empty(n_nodes, dtype=np.int64)
    lane = np.empty(n_nodes, dtype=np.int64)
    slot_ids[order] = snake
    lane[order] = rounds
    core = slot_ids // NDW
    dw = slot_ids % NDW
    reg = (dw >= REG_DWS[0]).astype(np.int64)
    loc = np.where(reg == 0, dw * P, REG_BASE[1] + (dw - DW0[1]) * P) + lane
    gpos = np.where(reg == 0,
                    core * REG_ROWS[0] + dw * P,
                    REG_GROWS[0] + core * REG_ROWS[1] + (dw - DW0[1]) * P) + lane

    # token fields
    d_core = core[edge_row]
    d_dw = dw[edge_row]
    d_part = lane[edge_row]
    s_g = gpos[edge_col]
    s_r1 = s_g >= REG_GROWS[0]
    rem_sw = np.where(s_r1, 3 + (s_g - REG_GROWS[0]) // 25600, s_g // 24576)
    rem_sloc = np.where(s_r1, (s_g - REG_GROWS[0]) % 25600, s_g % 24576)
    is_local = core[edge_col] == d_core
    t_sw = np.where(is_local, LOCAL_W, rem_sw)
    t_sloc = np.where(is_local, loc[edge_col], rem_sloc)
    t_val = np.asarray(edge_vals, dtype=np.float32)

    # group rank order = block order, block = (dwc, phase, sg)
    NGR = NDW * NSW
    g_local = d_dw * NSW + t_sw
    ranked_groups = []
    for dwc, phi in BLK_ORDER:
        for sgdws in SG_BLOCKS[dwc]:
            for w in PHASES[phi]:
                for dwi in sgdws:
                    ranked_groups.append(dwi * NSW + w)
    ranked_groups = np.asarray(ranked_groups)
    rank_of_g = np.empty(NGR, dtype=np.int64)
    rank_of_g[ranked_groups] = np.arange(NGR)

    # counts per (core, group); shared schedule = max over cores, padded to
    # 16 tokens per group (idx-layout granularity); each (block, window)
    # segment padded to 128 so gather runs stay column-aligned
    cnt = np.zeros((NCORES, NGR), dtype=np.int64)
    np.add.at(cnt, (d_core, g_local), 1)
    n16_g = -(-cnt.max(axis=0) // 16) * 16         # [NGR] natural order
    grp_off = np.zeros(NGR, dtype=np.int64)
    seg_layout = {}                                # (blk_idx, w) -> (off, ncols)
    cur = 0
    bi = 0
    blk_bounds = []                                # (off, ntok) per block
    for dwc, phi in BLK_ORDER:
        for sgdws in SG_BLOCKS[dwc]:
            blk_off = cur
            for w in PHASES[phi]:
                seg_off = cur
                for dwi in sgdws:
                    g = dwi * NSW + w
                    grp_off[g] = cur
                    cur += int(n16_g[g])
                cur = -(-cur // P) * P
                seg_layout[(bi, w)] = (seg_off, (cur - seg_off) // P)
            blk_bounds.append((blk_off, cur - blk_off))
            bi += 1
    toktot = int(cur)

    # within-group index per token (stable order)
    K = d_core * NGR + rank_of_g[g_local]
    perm = np.argsort(K, kind="stable")
    Ks = K[perm]
    starts = np.r_[0, np.flatnonzero(np.diff(Ks)) + 1]
    grp_start = starts[np.searchsorted(Ks[starts], Ks)]
    within_grp = np.arange(len(Ks)) - grp_start
    tgt = grp_off[g_local[perm]] + within_grp
    c_perm = d_core[perm]

    sloc_arr = np.zeros((NCORES, toktot), dtype=np.int16)
    val_arr = np.zeros((NCORES, toktot), dtype=np.float32)
    dloc_arr = np.zeros((NCORES, toktot), dtype=np.float32)
    sloc_arr[c_perm, tgt] = t_sloc[perm].astype(np.int16)
    val_arr[c_perm, tgt] = t_val[perm]
    dloc_arr[c_perm, tgt] = d_part[perm].astype(np.float32)

    # schedule: blocks in processing order; each block = one (dwc, phi, sg)
    sched = []
    gi = 0
    for dwc, phi in BLK_ORDER:
        for sgdws in SG_BLOCKS[dwc]:
            blk_off = int(off_ranked[gi])
            nch_dw = {d: 0 for d in sgdws}
            for w in PHASES[phi]:
                for dwi in sgdws:
                    nch_dw[dwi] += int(chunks_g[dwi * NSW + w])
            seen = {d: 0 for d in sgdws}
            per_sw = []
            for w in PHASES[phi]:
                metas = []   # (dw, tok_off, first, last)
                for dwi in sgdws:
                    g = dwi * NSW + w
                    assert ranked_groups[gi] == g, (dwc, phi, w, dwi, gi)
                    nch = int(chunks_g[g])
                    off = int(grp_off[g])
                    for ci in range(nch):
                        seen[dwi] += 1
                        metas.append((dwi, off + ci * P,
                                      seen[dwi] == 1,
                                      seen[dwi] == nch_dw[dwi]))
                    gi += 1
                per_sw.append((w, metas))
            blk_end = int(off_ranked[gi])
            sched.append(dict(dwc=dwc, phi=phi, dws=sgdws, off=blk_off,
                              ntok=blk_end - blk_off, per_sw=per_sw,
                              nch_dw=nch_dw))

    return dict(core=core, loc=loc, sloc=sloc_arr, val=val_arr, dloc=dloc_arr,
                toktot=toktot, sched=sched)


def _idx16_layout(sloc_row):
    """int16 token array -> dma_gather idxs layout [128, n/16] (8x replicated)."""
    n = sloc_row.shape[0]
    a = np.zeros((16, n // 16), np.int16)
    a[np.arange(n) % 16, np.arange(n) // 16] = sloc_row
    return np.tile(a, (8, 1))


def _pm_layout(arr_row):
    """token array -> [128, n/128] (token t at [t%128, t//128])."""
    n = arr_row.shape[0]
    a = np.zeros((P, n // P), arr_row.dtype)
    a[np.arange(n) % P, np.arange(n) // P] = arr_row
    return a


# ---------------- device kernel ----------------
def _build_kernel(num_layers, sched, toktot, S3, debug_outputs=False):
    nc = bacc.Bacc(None, target_bir_lowering=False, num_swdge_queues=4)
    f32 = mybir.dt.float32
    NB = BATCH // P
    max_blktok = max(b["ntok"] for b in sched)
    rg = [list(range(NCORES))]

    x0_shard = nc.dram_tensor("x0_shard", [SHARD, DIM], f32, kind="ExternalInput")
    tok_idx = nc.dram_tensor("tok_idx", [P, toktot // 16], mybir.dt.int16,
                             kind="ExternalInput")
    tok_val = nc.dram_tensor("tok_val", [P, toktot // P], f32, kind="ExternalInput")
    tok_dloc = nc.dram_tensor("tok_dloc", [P, toktot // P], f32, kind="ExternalInput")
    iota_in = nc.dram_tensor("iota_in", [P, P], mybir.dt.bfloat16,
                             kind="ExternalInput")
    bpr_u = nc.dram_tensor("bpr_u", [P, BATCH // 16], mybir.dt.int16, kind="ExternalInput")
    bpr_p = nc.dram_tensor("bpr_p", [P, BATCH // 16], mybir.dt.int16, kind="ExternalInput")
    bpr_n = nc.dram_tensor("bpr_n", [P, BATCH // 16], mybir.dt.int16, kind="ExternalInput")
    comp_idx = nc.dram_tensor("comp_idx", [P, S3 // 16], mybir.dt.int16,
                              kind="ExternalInput")
    reg_mult = nc.dram_tensor("reg_mult", [P, NDW], mybir.dt.float32,
                              kind="ExternalInput")
    reg_slots = nc.dram_tensor("reg_slots", [P, P // 16], mybir.dt.int16,
                               kind="ExternalInput")
    out_loss = nc.dram_tensor("out_loss", [1, 2], f32, kind="ExternalOutput")
    dbg = {}
    if debug_outputs:
        dbg["pooled_shard"] = nc.dram_tensor("pooled_shard_out", [SHARD, DIM],
                                             f32, kind="ExternalOutput")

    with tile.TileContext(nc) as tc:
        with (
            tc.tile_pool(name="persist", bufs=1) as pp,
            tc.tile_pool(name="idxs", bufs=2) as ipool,
            tc.tile_pool(name="gath", bufs=6) as gpool,
            tc.tile_pool(name="work", bufs=3) as wpool,
            tc.tile_pool(name="ys", bufs=1) as ypool,
            tc.tile_pool(name="bpr", bufs=1) as bpool,
            tc.tile_pool(name="psum", bufs=1, space="PSUM") as psum_pool,
            tc.tile_pool(name="dram", bufs=1, space="DRAM") as dram,
        ):
            with tc.tile_critical():
                nc.gpsimd.load_library(library_config.mlp)

            # per-layer tables, one DRAM tensor per region so Tile tracks
            # the two AG chunks as independent dependencies
            bf16 = mybir.dt.bfloat16
            tables = []
            for l in range(num_layers + 1):
                t0 = dram.tile([REG_GROWS[0], 2 * DIM], bf16, tag=f"table{l}r0")
                t1 = dram.tile([REG_GROWS[1], 2 * DIM], bf16, tag=f"table{l}r1")
                tables.append((t0, t1))
            shard_bounce = dram.tile([SHARD, 2 * DIM], bf16)
            pooled_bounce = dram.tile([SHARD, DIM], f32)
            comp_bounce = dram.tile([S3 + 1, DIM], f32)
            comp_table = dram.tile([NCORES * (S3 + 1), DIM], f32)

            iota = pp.tile([P, P], mybir.dt.bfloat16)
            nc.sync.dma_start(out=iota[:], in_=iota_in[:])

            accum = pp.tile([P, NDW * DIM], f32)
            nc.sync.dma_start(
                out=accum[:].rearrange("p (dw j) -> p dw j", j=DIM),
                in_=x0_shard[:].rearrange("(dw p) j -> p dw j", p=P))

            # x0 -> bounce -> per-region AllGather into table0
            for dwc in (0, 1):
                sl = slice(REG_BASE[dwc], REG_BASE[dwc] + REG_ROWS[dwc])
                nc.gpsimd.dma_start(out=shard_bounce[sl, 0:DIM],
                                    in_=x0_shard[sl, :])
                nc.gpsimd.collective_compute(
                    "AllGather", mybir.AluOpType.bypass, replica_groups=rg,
                    ins=[shard_bounce[sl, :].opt()],
                    outs=[tables[0][dwc][:].opt()])

            tok_val_t = pp.tile([P, toktot // P], f32)
            nc.sync.dma_start(out=tok_val_t[:], in_=tok_val[:])
            tok_dloc_t = pp.tile([P, toktot // P], f32)
            nc.sync.dma_start(out=tok_dloc_t[:], in_=tok_dloc[:])

            ones = pp.tile([P, 1], f32)
            nc.gpsimd.memset(ones[:], 1.0)

            # ---- L2-reg partial: multiplicity-weighted local square sum ----
            # reg = sum over sampled rows of |x0|^2 = sum_r mult(r) * |x0_r|^2.
            # Every sampled row is owned by exactly one core; accum == x0 at
            # this point. Partial travels as an extra row of the comp table's
            # AllGather (no extra collective, no gathers).
            reg_mult_t = pp.tile([P, NDW], f32)
            nc.sync.dma_start(out=reg_mult_t[:], in_=reg_mult[:])
            sq = ypool.tile([P, REG_DWS[1] * DIM], f32, tag="ysum1", name="sq")
            rr = pp.tile([P, 1], f32)
            base = 0
            for k, nd in enumerate((74, 73)):
                c0, c1 = base * DIM, (base + nd) * DIM
                nc.vector.tensor_tensor(out=sq[:, :nd * DIM],
                                        in0=accum[:, c0:c1],
                                        in1=accum[:, c0:c1],
                                        op=mybir.AluOpType.mult)
                r1 = pp.tile([P, 74], f32, tag="regr1", name="r1")
                nc.vector.tensor_reduce(
                    out=r1[:, :nd],
                    in_=sq[:, :nd * DIM].rearrange("p (d j) -> p d j", j=DIM),
                    axis=mybir.AxisListType.X, op=mybir.AluOpType.add)
                nc.vector.tensor_tensor(out=r1[:, :nd], in0=r1[:, :nd],
                                        in1=reg_mult_t[:, base:base + nd],
                                        op=mybir.AluOpType.mult)
                r2 = pp.tile([P, 1], f32, tag="regr2", name="r2")
                nc.vector.tensor_reduce(out=r2[:], in_=r1[:, :nd],
                                        axis=mybir.AxisListType.X,
                                        op=mybir.AluOpType.add)
                if k == 0:
                    nc.vector.tensor_copy(out=rr[:], in_=r2[:])
                else:
                    nc.vector.tensor_tensor(out=rr[:], in0=rr[:], in1=r2[:],
                                            op=mybir.AluOpType.add)
                base += nd
            reg_ps = psum_pool.tile([1, 1], f32, tag="ps0")
            nc.tensor.matmul(reg_ps[:], ones[:], rr[:], start=True, stop=True)
            regrow = pp.tile([1, DIM], f32)
            nc.gpsimd.memset(regrow[:], 0.0)
            nc.scalar.copy(out=regrow[:, 0:1], in_=reg_ps[:])

            # ---- layers ----
            for layer in range(1, num_layers + 1):
                src = tables[layer - 1]
                ysums = {}
                last_blk_of_dwc = {dwc: max(i for i, b in enumerate(sched)
                                            if b["dwc"] == dwc)
                                   for dwc in (0, 1)}
                for bi, blk in enumerate(sched):
                    dwc = blk["dwc"]
                    phi = blk["phi"]
                    sgdws = blk["dws"]
                    if dwc not in ysums:
                        ysums[dwc] = ypool.tile(
                            [P, REG_DWS[dwc] * DIM], f32, tag=f"ysum{dwc}",
                            name=f"ysum{dwc}")
                    ysum = ysums[dwc]
                    sg_idx = ipool.tile([P, max_blktok // 16],
                                        mybir.dt.int16, tag="sgidx")
                    nc.sync.dma_start(
                        out=sg_idx[:, : blk["ntok"] // 16],
                        in_=tok_idx[:, blk["off"] // 16:
                                    (blk["off"] + blk["ntok"]) // 16])
                    ptiles = {}
                    for w, metas in blk["per_sw"]:
                        if w == LOCAL_W:
                            src_win = shard_bounce[:, :]
                        else:
                            src_win = src[WIN_REG[w]][
                                WIN_LBASE[w]:WIN_LBASE[w] + WIN_SIZE[w], :]
                        i = 0
                        while i < len(metas):
                            run = metas[i:i + MAXCH]
                            ntok = len(run) * P
                            t0 = run[0][1]
                            g = gpool.tile([P, MAXCH, 2 * DIM],
                                           mybir.dt.bfloat16, tag="g")
                            locw = (t0 - blk["off"]) // 16
                            nc.gpsimd.dma_gather(
                                g[:, :len(run), :], src_win,
                                sg_idx[:, locw:locw + ntok // 16],
                                ntok, ntok, 2 * DIM, single_packet=False)
                            for ci, (dwi, tc0, first, last) in enumerate(run):
                                j = dwi - sgdws[0]
                                if dwi not in ptiles:
                                    ptiles[dwi] = psum_pool.tile(
                                        [P, DIM], f32, tag=f"ps{j}",
                                        name=f"pt{j}")
                                s = wpool.tile([P, P], mybir.dt.bfloat16,
                                               tag="S")
                                col = tc0 // P
                                nc.vector.tensor_scalar(
                                    out=s[:], in0=iota[:],
                                    scalar1=tok_dloc_t[:, col:col + 1],
                                    scalar2=tok_val_t[:, col:col + 1],
                                    op0=mybir.AluOpType.is_equal,
                                    op1=mybir.AluOpType.mult)
                                nc.tensor.matmul(
                                    ptiles[dwi][:], s[:], g[:, ci, 0:DIM],
                                    start=first, stop=last)
                            i += len(run)
                    for dwi in sgdws:
                        ysl = ysum[:, (dwi - DW0[dwc]) * DIM:
                                   (dwi - DW0[dwc] + 1) * DIM]
                        pt = ptiles.get(dwi)
                        if phi == 0:
                            if pt is None:
                                nc.gpsimd.memset(ysl, 0.0)
                            else:
                                nc.scalar.copy(out=ysl, in_=pt[:])
                        else:
                            if pt is not None:
                                nc.vector.tensor_tensor(
                                    out=ysl, in0=ysl, in1=pt[:],
                                    op=mybir.AluOpType.add)
                            if phi == 2:
                                nc.vector.tensor_tensor(
                                    out=accum[:, dwi * DIM:(dwi + 1) * DIM],
                                    in0=accum[:, dwi * DIM:(dwi + 1) * DIM],
                                    in1=ysl, op=mybir.AluOpType.add)
                    if phi == 2:
                        r0 = REG_BASE[dwc] + (sgdws[0] - DW0[dwc]) * P
                        r1_ = r0 + len(sgdws) * P
                        c0 = (sgdws[0] - DW0[dwc]) * DIM
                        c1 = (sgdws[-1] - DW0[dwc] + 1) * DIM
                        nc.gpsimd.dma_start(
                            out=shard_bounce[r0:r1_, 0:DIM].rearrange(
                                "(dw p) j -> p dw j", p=P),
                            in_=ysum[:, c0:c1].rearrange(
                                "p (dw j) -> p dw j", j=DIM))
                    if bi == last_blk_of_dwc[dwc] and layer < num_layers:
                        sl = slice(REG_BASE[dwc], REG_BASE[dwc] + REG_ROWS[dwc])
                        nc.gpsimd.collective_compute(
                            "AllGather", mybir.AluOpType.bypass,
                            replica_groups=rg,
                            ins=[shard_bounce[sl, :].opt()],
                            outs=[tables[layer][dwc][:].opt()])

            nc.vector.tensor_scalar_mul(accum[:], accum[:],
                                        1.0 / (num_layers + 1))
            nc.sync.dma_start(
                out=pooled_bounce[:].rearrange("(dw p) j -> p dw j", p=P),
                in_=accum[:].rearrange("p (dw j) -> p dw j", j=DIM))
            if debug_outputs:
                nc.sync.dma_start(out=dbg["pooled_shard"][:], in_=pooled_bounce[:])

            # ---- BPR loss1: compact local pooled rows, AllGather, gather ----
            comp_idx_t = pp.tile([P, S3 // 16], mybir.dt.int16, tag="compidx")
            nc.sync.dma_start(out=comp_idx_t[:], in_=comp_idx[:])
            ct = bpool.tile([P, S3 // P, DIM], f32, tag="compt")
            o = 0
            while o < S3:
                n = min(2048, S3 - o)
                nc.gpsimd.dma_gather(
                    ct[:, o // P:(o + n) // P, :], pooled_bounce[:],
                    comp_idx_t[:, o // 16:(o + n) // 16],
                    n, n, DIM, single_packet=False)
                o += n
            nc.sync.dma_start(
                out=comp_bounce[0:S3, :].rearrange("(b p) j -> p b j", p=P),
                in_=ct[:])
            nc.sync.dma_start(out=comp_bounce[S3:S3 + 1, :], in_=regrow[:])
            nc.gpsimd.collective_compute(
                "AllGather", mybir.AluOpType.bypass, replica_groups=rg,
                ins=[comp_bounce[:].opt()], outs=[comp_table[:].opt()])

            bidx = {}
            for name, srct in (("u", bpr_u), ("p", bpr_p), ("n", bpr_n)):
                t = pp.tile([P, BATCH // 16], mybir.dt.int16, tag=f"bidx{name}")
                nc.sync.dma_start(out=t[:], in_=srct[:])
                bidx[name] = t

            HB = BATCH // 2          # triples per half-pass
            HNB = HB // P

            def bpr_gather(idx_tile, h, tag):
                out_t = bpool.tile([P, HNB, DIM], f32, tag=tag, name="bg")
                o = 0
                while o < HB:
                    n = min(2048, HB - o)
                    oo = h * HB + o
                    nc.gpsimd.dma_gather(
                        out_t[:, o // P:(o + n) // P, :], comp_table[:],
                        idx_tile[:, oo // 16:(oo + n) // 16],
                        n, n, DIM, single_packet=False)
                    o += n
                return out_t

            ps = pp.tile([P, NB], f32, tag="psc")
            ns = pp.tile([P, NB], f32, tag="nsc")
            for h in (0, 1):
                U = bpr_gather(bidx["u"], h, "bgU")
                Pp = bpr_gather(bidx["p"], h, "bgV")
                tmp = bpool.tile([P, HNB, DIM], f32, tag="tmp")
                nc.vector.tensor_tensor(out=tmp[:], in0=U[:], in1=Pp[:],
                                        op=mybir.AluOpType.mult)
                nc.vector.tensor_reduce(out=ps[:, h * HNB:(h + 1) * HNB],
                                        in_=tmp[:],
                                        axis=mybir.AxisListType.X,
                                        op=mybir.AluOpType.add)
                Nn = bpr_gather(bidx["n"], h, "bgV")
                nc.vector.tensor_tensor(out=tmp[:], in0=U[:], in1=Nn[:],
                                        op=mybir.AluOpType.mult)
                nc.vector.tensor_reduce(out=ns[:, h * HNB:(h + 1) * HNB],
                                        in_=tmp[:],
                                        axis=mybir.AxisListType.X,
                                        op=mybir.AluOpType.add)
            d = pp.tile([P, NB], f32, tag="dsc")
            nc.vector.tensor_tensor(out=d[:], in0=ns[:], in1=ps[:],
                                    op=mybir.AluOpType.subtract)
            # softplus(d) = ln(1 + exp(d)) — Softplus has no ACT table here
            sp = pp.tile([P, NB], f32, tag="spc")
            nc.scalar.activation(sp[:], d[:], mybir.ActivationFunctionType.Exp)
            nc.vector.tensor_scalar_add(sp[:], sp[:], 1.0)
            nc.scalar.activation(sp[:], sp[:], mybir.ActivationFunctionType.Ln)
            s1 = pp.tile([P, 1], f32)
            nc.vector.tensor_reduce(out=s1[:], in_=sp[:],
                                    axis=mybir.AxisListType.X,
                                    op=mybir.AluOpType.add)
            loss_ps = psum_pool.tile([1, 1], f32, tag="ps1")
            nc.tensor.matmul(loss_ps[:], ones[:], s1[:], start=True, stop=True)

            # reg finalize: the 8 per-core partials sit at comp-table row
            # c*(S3+1)+S3 col 0; gather them (16x replicated to 128 idxs),
            # partition-sum via ones-matmul, scale by /16 for the replication.
            reg_slots_t = pp.tile([P, P // 16], mybir.dt.int16, tag="regslots")
            nc.sync.dma_start(out=reg_slots_t[:], in_=reg_slots[:])
            gr = bpool.tile([P, 1, DIM], f32, tag="greg")
            nc.gpsimd.dma_gather(gr[:], comp_table[:], reg_slots_t[:],
                                 P, P, DIM, single_packet=False)
            reg_fin = psum_pool.tile([1, 1], f32, tag="ps2")
            nc.tensor.matmul(reg_fin[:], ones[:], gr[:, 0, 0:1],
                             start=True, stop=True)

            tot = pp.tile([1, 2], f32)
            nc.vector.tensor_scalar_mul(tot[:, 0:1], loss_ps[:], 1.0 / BATCH)
            nc.vector.tensor_scalar_mul(tot[:, 1:2], reg_fin[:],
                                        0.5 / BATCH / 16.0)
            nc.sync.dma_start(out=out_loss[:], in_=tot[:])

    nc.compile()
    _spread_swdge_queues(nc)
    return nc


def _spread_swdge_queues(nc, nq=4):
    """Post-schedule: route each SWDGE op to queue (assigned DMASW lane % nq).

    Tile assigns DMASW completion-sem lanes round-robin over SWDGE ops in
    scheduled order; pairing queue = lane % nq keeps each sem lane locked to
    one queue (required for in-order completion semantics) while spreading
    work over all 4 HW SWDGE queues (~3x gather throughput).
    """
    import re
    pat = re.compile(r"DMASW(\d+)_")
    for bb in nc.main_func.blocks:
        for ins in bb.instructions:
            tn = type(ins).__name__
            if tn not in ("InstDMAGatherAnt", "InstDMACopy"):
                continue
            if tn == "InstDMACopy" and getattr(ins, "queue", None) is not None \
                    and not str(ins.queue).startswith("qPoolDynamic"):
                continue
            if tn == "InstDMACopy" and getattr(ins, "queue", None) is None:
                continue
            si = ins.sync_info
            if not si or not si.on_update:
                continue
            m = pat.match(si.on_update[0].ant_name or "")
            if not m:
                continue
            q = int(m.group(1)) % nq
            if tn == "InstDMAGatherAnt":
                ins.queue_num = q
            else:
                ins.queue = f"qPoolDynamic{q if q else ''}"


# ---------------- public entry point ----------------
def build_for_sim(user_weight, item_weight, edge_vals, edge_row, edge_col,
                  user_index, pos_index, neg_index, num_layers, _debug=False):
    """Build the compiled module + per-core input maps without executing."""
    return _prepare(user_weight, item_weight, edge_vals, edge_row, edge_col,
                    user_index, pos_index, neg_index, num_layers, _debug)


def _prepare(user_weight, item_weight, edge_vals, edge_row, edge_col,
             user_index, pos_index, neg_index, num_layers, _debug=False):
    user_weight = np.asarray(user_weight, dtype=np.float32)
    item_weight = np.asarray(item_weight, dtype=np.float32)
    edge_vals = np.asarray(edge_vals, dtype=np.float32)
    edge_row = np.asarray(edge_row, dtype=np.int64)
    edge_col = np.asarray(edge_col, dtype=np.int64)
    user_index = np.asarray(user_index, dtype=np.int64)
    pos_index = np.asarray(pos_index, dtype=np.int64)
    neg_index = np.asarray(neg_index, dtype=np.int64)
    L = int(num_layers)

    pre = _preprocess(edge_row, edge_col, edge_vals)
    core_n, loc_n = pre["core"], pre["loc"]

    x0_nodes = np.concatenate([user_weight, item_weight], axis=0)
    x0_shards = np.zeros((NCORES, SHARD, DIM), np.float32)
    x0_shards[core_n, loc_n] = x0_nodes

    iota = np.tile(np.arange(P, dtype=np.float32),
                   (P, 1)).astype(ml_dtypes.bfloat16)

    # BPR pooled-side compaction: per-core unique local rows referenced
    node_u = user_index
    node_p = NUM_USERS + pos_index
    node_n = NUM_USERS + neg_index
    trip_nodes = {"u": node_u, "p": node_p, "n": node_n}
    allk = np.unique(np.concatenate(
        [core_n[v] * SHARD + loc_n[v] for v in trip_nodes.values()]))
    core_of = allk // SHARD
    within = allk % SHARD
    uniq = [within[core_of == c] for c in range(NCORES)]
    S3 = max(128, -(-max(len(x) for x in uniq) // 128) * 128)
    assert NCORES * (S3 + 1) < 32768, f"BPR compact table too large: {S3}"
    comp_idx_arr = []
    slot_map = np.zeros(NCORES * SHARD, dtype=np.int64)
    for c in range(NCORES):
        u_c = uniq[c]
        pad = np.zeros(S3, np.int64)
        pad[:len(u_c)] = u_c
        comp_idx_arr.append(_idx16_layout(pad.astype(np.int16)))
        slot_map[c * SHARD + u_c] = c * (S3 + 1) + np.arange(len(u_c))
    b_tok = {k: _idx16_layout(
        slot_map[core_n[v] * SHARD + loc_n[v]].astype(np.int16))
        for k, v in trip_nodes.items()}

    # L2-reg multiplicities: count of each local row among the 3*BATCH samples
    all_nodes = np.concatenate([node_u, node_p, node_n])
    M = np.zeros((NCORES, SHARD), np.float32)
    np.add.at(M, (core_n[all_nodes], loc_n[all_nodes]), 1.0)
    reg_mult_arr = [M[c].reshape(NDW, P).T.copy() for c in range(NCORES)]
    reg_slot_list = np.array([c * (S3 + 1) + S3 for c in range(NCORES)],
                             np.int64)
    reg_slots_arr = _idx16_layout(
        np.tile(reg_slot_list, P // NCORES).astype(np.int16))

    nc = _build_kernel(L, pre["sched"], pre["toktot"], S3,
                       debug_outputs=_debug)

    in_maps = []
    for c in range(NCORES):
        in_maps.append({
            "x0_shard": x0_shards[c],
            "tok_idx": _idx16_layout(pre["sloc"][c]),
            "tok_val": _pm_layout(pre["val"][c]),
            "tok_dloc": _pm_layout(pre["dloc"][c]),
            "iota_in": iota,
            "bpr_u": b_tok["u"], "bpr_p": b_tok["p"], "bpr_n": b_tok["n"],
            "comp_idx": comp_idx_arr[c],
            "reg_mult": reg_mult_arr[c],
            "reg_slots": reg_slots_arr,
        })
    _prepare.last_maps = (core_n, loc_n)
    return nc, in_maps


def kernel(user_weight, item_weight, edge_vals, edge_row, edge_col,
           user_index, pos_index, neg_index, num_layers, _debug=False):
    nc, in_maps = _prepare(user_weight, item_weight, edge_vals, edge_row,
                           edge_col, user_index, pos_index, neg_index,
                           num_layers, _debug)
    from concourse.bass_utils import run_bass_kernel_spmd
    kernel._cache = (nc, in_maps)
    res = run_bass_kernel_spmd(nc, in_maps, core_ids=list(range(NCORES)))
    out = res.results[0]["out_loss"]
    loss1 = np.float32(out[0, 0])
    reg = np.float32(out[0, 1])
    if _debug:
        pooled = np.stack(
            [res.results[c]["pooled_shard_out"] for c in range(NCORES)], axis=0)
        kernel._debug_pooled = (pooled, _prepare.last_maps)
    return loss1, reg
